# revision 27
# baseline (speedup 1.0000x reference)
"""Trainium2 distributed FPS (farthest-point-sampling) signature kernel.

Problem: nn_ApproximatePHProbe — greedy maxmin landmark sampling of 256
landmarks from 500k 64-d points, then top-10 eigenvalues of the landmark
distance matrix.

Device part (8 NeuronCores, SPMD):
  - points data-parallel sharded along N; each core keeps its whole shard
    resident in SBUF as a matmul-weight layout (2 points packed per
    128-row weight column).
  - each FPS iteration: one sweep of 245 fp32 matmuls computes -2*x.p for
    all local points into PSUM [128, 490]; a DVE chain forms the new
    squared distances (xx + pp - 2 x.p), min-updates the running
    min-distance field, and extracts (maxval, first-argmax-index) exactly;
    candidates are exchanged with a tiny AllGather; every core
    deterministically selects the global winner (max val, ties -> lowest
    core, lowest index) and rebuilds the next query from the gathered
    winner coordinates via mask-matmuls.
  - device output: 255 winner (core, local-index) pairs.
Host part: prepend landmark 0, gather exact f32 coords, build the 256x256
distance matrix, eigvalsh, signature + rupture decision (tiny).
"""

import os
import numpy as np

import concourse.bass as bass
import concourse.bacc as bacc
import concourse.mybir as mybir
import concourse.tile as tile
from concourse.bass_utils import run_bass_kernel_spmd

F32 = mybir.dt.float32
BF16 = mybir.dt.bfloat16
I32 = mybir.dt.int32

N_CORES = 8
DIM = 64
NUM_LANDMARKS = 256
SIGNATURE_SIZE = 10
REL_THRESH = 0.2
BIG = float(2**20)

# Payload layout (per-core candidate row, f32):
#   [0]     xx  (|x|^2 of candidate)
#   [1:65]  x   (candidate coords)
#   [65]    j   (local index, as float)
#   [66]    val (candidate's min-dist^2 = local max)
#   [67:72] pad
PAY = 72
PC_XX, PC_X0, PC_J, PC_VAL = 0, 1, 65, 66


def _pack_layout(arr_pts, chunks, dtype=np.float32):
    """[n_pad, 64] -> weight layout [128, chunks*128].

    Weight column (c*128 + m) holds [x_{256c+2m}(64); x_{256c+2m+1}(64)].
    """
    v = arr_pts.reshape(chunks, 128, 2, DIM)  # [c, m, q, d]
    return np.ascontiguousarray(
        v.transpose(2, 3, 0, 1).reshape(2 * DIM, chunks * 128).astype(dtype)
    )


def _col_layout(arr, chunks):
    """[n_pad] -> psum-matching layout [128, 2*chunks].

    psum[m, 2c+q] corresponds to local point j = 256c + 2m + q.
    """
    v = arr.reshape(chunks, 128, 2)  # [c, m, q]
    return np.ascontiguousarray(v.transpose(1, 0, 2).reshape(128, 2 * chunks))


def build_fps_nc(n_pad, chunks, num_it, dyn_gather=True, stage=9, wdtype=BF16):
    """Build the SPMD Bass graph (identical on all 8 cores).

    stage (debug bisect): 1=sweep+reduce, 2=+argmax-index, 3=+payload,
    4=+collective, 5=+select+winner, 9=full.
    """
    cols = 2 * chunks
    wcols = 128 * chunks
    nc = bacc.Bacc("TRN2", num_devices=N_CORES)

    xw_d = nc.declare_dram_parameter("xw", [128, wcols], F32, isOutput=False)
    xxl_d = nc.declare_dram_parameter("xxl", [128, cols], F32, isOutput=False)
    iob_d = nc.declare_dram_parameter("iob", [128, cols], F32, isOutput=False)
    ms_d = nc.declare_dram_parameter("mseed", [128, cols], F32, isOutput=False)
    xaug_d = nc.declare_dram_parameter("xaug", [n_pad, PAY], F32, isOutput=False)
    rhs0_d = nc.declare_dram_parameter("rhs0", [128, 2], F32, isOutput=False)
    pp0_d = nc.declare_dram_parameter("pp0", [1, 1], F32, isOutput=False)
    cio_d = nc.declare_dram_parameter("cion", [1, 8], F32, isOutput=False)
    ci8_d = nc.declare_dram_parameter("ciota8", [8, 1], F32, isOutput=False)
    out_d = nc.declare_dram_parameter("out", [4, num_it], F32, isOutput=True)

    ident_dram = nc.inline_tensor(np.eye(128, dtype=np.float32), name="ident")
    ones_dram = nc.inline_tensor(np.ones((4, 128), dtype=np.float32), name="ones4")

    rg = [list(range(N_CORES))]

    with tile.TileContext(nc) as tc:
        with (
            tc.tile_pool(name="persist", bufs=1) as pp_pool,
            tc.tile_pool(name="scratch", bufs=2) as sp,
            tc.tile_pool(name="pd", bufs=1, space="PSUM") as pd_pool,
            tc.tile_pool(name="pssmall", bufs=2, space="PSUM") as ps,
            tc.tile_pool(name="psjunk", bufs=1, space="PSUM") as psj,
            tc.tile_pool(name="dram", bufs=2, space="DRAM") as dp,
        ):
            # ---- persistent state ----
            xw = pp_pool.tile([128, wcols], wdtype, tag="xw")
            xstage = pp_pool.tile([128, wcols // 8], F32, tag="xstage")
            xxl = pp_pool.tile([128, cols], F32, tag="xxl")
            iob = pp_pool.tile([128, cols], F32, tag="iob")
            m = pp_pool.tile([128, cols], F32, tag="m")
            ident = pp_pool.tile([128, 128], F32, tag="ident")
            ones4 = pp_pool.tile([4, 128], F32, tag="ones4")
            rhs = pp_pool.tile([128, 2], wdtype, tag="rhs")
            ppb = pp_pool.tile([128, 1], F32, tag="ppb")
            cio = pp_pool.tile([1, 8], F32, tag="cio")
            ci8 = pp_pool.tile([8, 1], F32, tag="ci8")
            winners_a = pp_pool.tile([1, num_it], F32, tag="winners_a")
            winners_b = pp_pool.tile([3, num_it], F32, tag="winners_b")

            # ---- preamble DMAs (all on the single gpsimd SWDGE queue so
            # downstream matmuls see at most one DMA wait) ----
            # xw arrives f32 and is cast on-device to the sweep dtype in
            # 8 staged chunks (bf16 external inputs trip the pjrt path).
            wstep = wcols // 8
            for i in range(8):
                nc.gpsimd.dma_start(
                    xstage[:, 0:wstep], xw_d[:, i * wstep:(i + 1) * wstep]
                )
                nc.vector.tensor_copy(
                    xw[:, i * wstep:(i + 1) * wstep], xstage[:, 0:wstep]
                )
            nc.gpsimd.dma_start(xxl[:, :], xxl_d[:, :])
            nc.gpsimd.dma_start(iob[:, :], iob_d[:, :])
            nc.gpsimd.dma_start(m[:, :], ms_d[:, :])
            nc.gpsimd.dma_start(ident[:, :], ident_dram[:, :])
            nc.gpsimd.dma_start(ones4[:, :], ones_dram[:, :])
            rhs0_sb = pp_pool.tile([128, 2], F32, tag="rhs0sb")
            nc.gpsimd.dma_start(rhs0_sb[:, :], rhs0_d[:, :])
            nc.vector.tensor_copy(rhs[:, :], rhs0_sb[:, :])
            nc.gpsimd.dma_start(cio[:, :], cio_d[:, :])
            nc.gpsimd.dma_start(ci8[:, :], ci8_d[:, :])
            nc.gpsimd.memset(winners_a[:, :], 0.0)
            nc.gpsimd.memset(winners_b[:, :], 0.0)

            pp0_sb = pp_pool.tile([1, 1], F32, tag="pp0")
            nc.gpsimd.dma_start(pp0_sb[:, :], pp0_d[:, :])

            # PE wait-absorber ladder: a matmul instruction supports only ONE
            # sync-wait, so teach the PE each input DMA queue's tick via junk
            # matmuls that each depend on a single tile.
            ps_junk = psj.tile([128, 1], F32, tag="ps_junk")
            for ap in (xw[:, 0:1], rhs[:, 0:1], ident[:, 0:1]):
                nc.tensor.matmul(ps_junk[0:1, 0:1], lhsT=ap, rhs=ap)
            nc.tensor.matmul(
                ps_junk[0:1, 0:1], lhsT=ones4[0:1, 0:1], rhs=ones4[0:1, 0:1]
            )
            nc.tensor.matmul(ps_junk[0:1, 0:1], lhsT=pp0_sb[:, :], rhs=pp0_sb[:, :])

            # ppb <- broadcast(pp0)
            ppb_ps0 = ps.tile([128, 1], F32, tag="ps_a")
            nc.tensor.matmul(ppb_ps0[:, :], lhsT=ones4[0:1, :], rhs=pp0_sb[:, :])
            nc.vector.tensor_copy(ppb[:, :], ppb_ps0[:, :])

            greg = nc.gpsimd.alloc_register("goff")

            for k in range(num_it if stage >= 1 else 0):
                # ---------- sweep: psum[m, 2c+q] = -2 * x_{256c+2m+q} . p ----
                pd = pd_pool.tile([128, cols], F32, tag="pd")
                for c in range(chunks):
                    nc.tensor.matmul(
                        pd[:, 2 * c:2 * c + 2],
                        lhsT=xw[:, 128 * c:128 * (c + 1)],
                        rhs=rhs[:, 0:2],
                        start=True,
                        stop=True,
                    )

                # ---------- nd = pd + pp + xx ; m = min(m, nd); mx = rowmax --
                if stage < 1.2:
                    continue
                t1 = sp.tile([128, cols], F32, tag="t1")
                nc.vector.tensor_scalar(
                    out=t1[:, :], in0=pd[:, :], scalar1=ppb[:, 0:1], scalar2=None,
                    op0=mybir.AluOpType.add,
                )
                if stage < 1.4:
                    continue
                nc.vector.tensor_tensor(
                    out=t1[:, :], in0=t1[:, :], in1=xxl[:, :],
                    op=mybir.AluOpType.add,
                )
                if stage < 1.6:
                    continue
                mx = sp.tile([128, 1], F32, tag="mx")
                nc.vector.tensor_tensor(
                    out=m[:, :], in0=m[:, :], in1=t1[:, :],
                    op=mybir.AluOpType.min,
                )
                nc.vector.tensor_reduce(
                    out=mx[:, :], in_=m[:, :],
                    axis=mybir.AxisListType.X, op=mybir.AluOpType.max,
                )

                if stage < 2:
                    continue
                # ---------- global (per-core) max ---------------------------
                pmxT = ps.tile([1, 128], F32, tag="ps_a")
                nc.tensor.matmul(pmxT[:, :], lhsT=mx[:, 0:1], rhs=ident[:, :])
                gmax = sp.tile([1, 1], F32, tag="gmax")
                nc.vector.tensor_reduce(
                    out=gmax[:, :], in_=pmxT[:, :],
                    axis=mybir.AxisListType.X, op=mybir.AluOpType.max,
                )
                pg = ps.tile([128, 1], F32, tag="ps_b")
                nc.tensor.matmul(pg[:, :], lhsT=ones4[0:1, :], rhs=gmax[:, :])
                gb = sp.tile([128, 1], F32, tag="gb")
                nc.vector.tensor_copy(gb[:, :], pg[:, :])

                # ---------- first index attaining the max -------------------
                e = sp.tile([128, cols], F32, tag="e")
                nc.vector.tensor_scalar(
                    out=e[:, :], in0=m[:, :], scalar1=gb[:, 0:1], scalar2=None,
                    op0=mybir.AluOpType.is_equal,
                )
                rmin = sp.tile([128, 1], F32, tag="rmin")
                nc.vector.tensor_tensor(
                    out=e[:, :], in0=e[:, :], in1=iob[:, :],
                    op=mybir.AluOpType.mult,
                )
                nc.vector.tensor_reduce(
                    out=rmin[:, :], in_=e[:, :],
                    axis=mybir.AxisListType.X, op=mybir.AluOpType.min,
                )
                prT = ps.tile([1, 128], F32, tag="ps_a")
                nc.tensor.matmul(prT[:, :], lhsT=rmin[:, 0:1], rhs=ident[:, :])
                jenc = sp.tile([1, 1], F32, tag="jenc")
                nc.vector.tensor_reduce(
                    out=jenc[:, :], in_=prT[:, :],
                    axis=mybir.AxisListType.X, op=mybir.AluOpType.min,
                )
                jf = sp.tile([1, 1], F32, tag="jf")
                nc.vector.tensor_scalar(
                    out=jf[:, :], in0=jenc[:, :], scalar1=BIG, scalar2=None,
                    op0=mybir.AluOpType.add,
                )
                ji = sp.tile([1, 1], I32, tag="ji")
                nc.vector.tensor_copy(ji[:, :], jf[:, :])

                if stage < 3:
                    continue
                # ---------- payload + AllGather -----------------------------
                pay = sp.tile([1, PAY], F32, tag="pay")
                nc.gpsimd.memset(pay[0:1, PC_VAL + 1:PAY], 0.0)
                if dyn_gather:
                    nc.gpsimd.reg_load(greg, ji[0:1, 0:1])
                    goff = nc.gpsimd.snap(greg)
                    nc.gpsimd.dma_start(
                        pay[0:1, 0:PC_J], xaug_d[bass.ds(goff, 1), 0:PC_J]
                    )
                else:
                    nc.gpsimd.dma_start(pay[0:1, 0:PC_J], xaug_d[0:1, 0:PC_J])
                nc.vector.tensor_copy(pay[0:1, PC_J:PC_J + 1], jf[:, :])
                nc.vector.tensor_copy(pay[0:1, PC_VAL:PC_VAL + 1], gmax[:, :])

                if stage < 4:
                    continue
                ccin = dp.tile([1, PAY], F32, tag="ccin")
                ccout = dp.tile([8, PAY], F32, tag="ccout")
                nc.sync.dma_start(ccin[:, :], pay[:, :])
                nc.gpsimd.collective_compute(
                    "AllGather",
                    mybir.AluOpType.bypass,
                    replica_groups=rg,
                    ins=[ccin[:, :].opt()],
                    outs=[ccout[:, :].opt()],
                )
                agb0 = sp.tile([8, PAY], F32, tag="agb0")
                nc.sync.dma_start(agb0[:, :], ccout[:, :])
                # DVE-side copy so PE select-matmuls depend on a single
                # (vector-engine) semaphore, not the DMA queue.
                agb = sp.tile([8, PAY], F32, tag="agb")
                nc.vector.tensor_copy(agb[:, :], agb0[:, :])
                vrow = sp.tile([1, 8], F32, tag="vrow")
                nc.sync.dma_start(
                    vrow[0:1, 0:8], ccout[0:8, PC_VAL:PC_VAL + 1]
                )

                if stage < 5:
                    continue
                # ---------- winning core (max val, ties -> lowest core) -----
                g8 = sp.tile([1, 1], F32, tag="g8")
                nc.vector.tensor_reduce(
                    out=g8[:, :], in_=vrow[:, :],
                    axis=mybir.AxisListType.X, op=mybir.AluOpType.max,
                )
                e8 = sp.tile([1, 8], F32, tag="e8")
                nc.vector.tensor_scalar(
                    out=e8[:, :], in0=vrow[:, :], scalar1=g8[:, 0:1], scalar2=None,
                    op0=mybir.AluOpType.is_equal,
                )
                nc.vector.tensor_tensor(
                    out=e8[:, :], in0=e8[:, :], in1=cio[:, :],
                    op=mybir.AluOpType.mult,
                )
                ce = sp.tile([1, 1], F32, tag="ce")
                nc.vector.tensor_reduce(
                    out=ce[:, :], in_=e8[:, :],
                    axis=mybir.AxisListType.X, op=mybir.AluOpType.min,
                )
                cf = sp.tile([1, 1], F32, tag="cf")
                nc.vector.tensor_scalar(
                    out=cf[:, :], in0=ce[:, :], scalar1=8.0, scalar2=None,
                    op0=mybir.AluOpType.add,
                )

                # ---------- exclusive one-hot core mask ---------------------
                pc8 = ps.tile([8, 1], F32, tag="ps_b")
                nc.tensor.matmul(pc8[:, :], lhsT=ones4[0:1, 0:8], rhs=cf[:, :])
                mask = sp.tile([8, 1], F32, tag="mask")
                nc.vector.tensor_tensor(
                    out=mask[:, :], in0=ci8[:, :], in1=pc8[:, :],
                    op=mybir.AluOpType.is_equal,
                )
                maskm2 = sp.tile([8, 1], F32, tag="maskm2")
                nc.vector.tensor_scalar(
                    out=maskm2[:, :], in0=mask[:, :], scalar1=-2.0, scalar2=None,
                    op0=mybir.AluOpType.mult,
                )

                # ---------- select winner row via mask-matmuls --------------
                prhs = ps.tile([128, 2], F32, tag="ps_c")
                nc.tensor.matmul(
                    prhs[0:64, 0:1],
                    lhsT=agb[0:8, PC_X0:PC_X0 + DIM], rhs=maskm2[:, 0:1],
                )
                nc.tensor.matmul(
                    prhs[64:128, 1:2],
                    lhsT=agb[0:8, PC_X0:PC_X0 + DIM], rhs=maskm2[:, 0:1],
                )
                pmisc = ps.tile([3, 1], F32, tag="ps_b")
                nc.tensor.matmul(
                    pmisc[:, :], lhsT=agb[0:8, PC_J - 1:PC_J + 2], rhs=mask[:, 0:1]
                )
                misc = sp.tile([3, 1], F32, tag="misc")
                nc.vector.tensor_copy(misc[:, :], pmisc[:, :])
                pxx = ps.tile([1, 1], F32, tag="ps_b")
                nc.tensor.matmul(
                    pxx[:, :], lhsT=agb[0:8, PC_XX:PC_XX + 1], rhs=mask[:, 0:1]
                )
                xxw = sp.tile([1, 1], F32, tag="xxw")
                nc.vector.tensor_copy(xxw[:, :], pxx[:, :])

                # ---------- update loop state -------------------------------
                nc.vector.tensor_copy(rhs[0:64, 0:1], prhs[0:64, 0:1])
                nc.vector.tensor_copy(rhs[64:128, 1:2], prhs[64:128, 1:2])
                pppb = ps.tile([128, 1], F32, tag="ps_a")
                nc.tensor.matmul(pppb[:, :], lhsT=ones4[0:1, :], rhs=xxw[:, :])
                nc.vector.tensor_copy(ppb[:, :], pppb[:, :])

                # ---------- record winner -----------------------------------
                nc.vector.tensor_copy(winners_a[0:1, k:k + 1], cf[:, :])
                nc.vector.tensor_copy(winners_b[0:3, k:k + 1], misc[0:3, 0:1])

            nc.sync.dma_start(out_d[0:1, :], winners_a[:, :])
            nc.sync.dma_start(out_d[1:4, :], winners_b[:, :])

    nc.finalize()
    return nc


def _host_prep(points, n_cores, n_pad, chunks):
    """Build per-core input dicts."""
    n_total, dim = points.shape
    n_per = n_total // n_cores
    cols = 2 * chunks
    in_maps = []
    p0 = points[0]
    pp0 = np.float32(np.dot(p0, p0))
    rhs0 = np.zeros((128, 2), dtype=np.float32)
    rhs0[0:64, 0] = -2.0 * p0
    rhs0[64:128, 1] = -2.0 * p0
    cion = (np.arange(8, dtype=np.float32) - 8.0).reshape(1, 8)
    ciota8 = np.arange(8, dtype=np.float32).reshape(8, 1)

    for c in range(n_cores):
        pts = np.zeros((n_pad, dim), dtype=np.float32)
        pts[0:n_per] = points[c * n_per:(c + 1) * n_per]
        xx = np.einsum("ij,ij->i", pts, pts).astype(np.float32)
        jj = np.arange(n_pad, dtype=np.float32)
        mseed = np.where(jj < n_per, 1.0e30, -1.0e30).astype(np.float32)

        xaug = np.zeros((n_pad, PAY), dtype=np.float32)
        xaug[:, 0] = xx
        xaug[:, 1:1 + dim] = pts

        in_maps.append({
            "xw": _pack_layout(pts, chunks),
            "xxl": _col_layout(xx, chunks),
            "iob": _col_layout(jj - BIG, chunks),
            "mseed": _col_layout(mseed, chunks),
            "xaug": xaug,
            "rhs0": rhs0,
            "pp0": pp0.reshape(1, 1),
            "cion": cion,
            "ciota8": ciota8,
        })
    return in_maps


def _signature_from_landmarks(lm):
    """Final ~20 lines of the reference, in numpy (lm: [256, 64] f32)."""
    sq = ((lm[:, None, :] - lm[None, :, :]) ** 2).sum(-1).astype(np.float32)
    D = np.where(sq > 0, np.sqrt(np.where(sq > 0, sq, np.float32(1.0))), 0.0)
    D = D.astype(np.float32)
    D = ((D + D.T) * np.float32(0.5)
         + np.eye(NUM_LANDMARKS, dtype=np.float32) * np.float32(1e-6))
    ev = np.linalg.eigvalsh(D)[::-1][:SIGNATURE_SIZE].astype(np.float32)
    return (ev / (np.abs(ev[0]) + np.float32(1e-8))).astype(np.float32)


_NC_CACHE = {}


def run_fps_device(points, num_it, n_pad, chunks, trace=False):
    key = (num_it, n_pad, chunks)
    if key not in _NC_CACHE:
        _NC_CACHE[key] = build_fps_nc(n_pad, chunks, num_it)
    nc = _NC_CACHE[key]
    in_maps = _host_prep(points, N_CORES, n_pad, chunks)
    res = run_bass_kernel_spmd(
        nc, in_maps, core_ids=list(range(N_CORES)), trace=trace,
        tmpdir="/tmp/fps_trace" if trace else None,
    )
    # out rows: 0 = winning core, 1 = (junk), 2 = local index, 3 = value
    out0 = np.asarray(res.results[0]["out"])
    cs = out0[0, :].astype(np.int64)
    js = out0[2, :].astype(np.int64)
    n_per = points.shape[0] // N_CORES
    gidx = np.concatenate([np.zeros(1, dtype=np.int64), cs * n_per + js])
    return gidx, res


def kernel(points, prev_sig, step_count):
    points = np.asarray(points, dtype=np.float32)
    prev_sig = np.asarray(prev_sig, dtype=np.float32)

    # 500000/8 = 62500 per core, padded to 62720 = 245 chunks * 256 points
    gidx, _ = run_fps_device(points, num_it=NUM_LANDMARKS - 1,
                             n_pad=62720, chunks=245)
    lm = points[gidx]
    sig = _signature_from_landmarks(lm)

    first = int(step_count) == 0
    if first:
        rel = np.float32(0.0)
    else:
        delta = np.abs(sig - prev_sig).astype(np.float32)
        rel = np.float32(delta.sum() / (np.abs(prev_sig).sum() + np.float32(1e-8)))
    is_rupture = np.bool_((not first) and (rel > np.float32(REL_THRESH)))
    return np.array(is_rupture), np.array(rel, dtype=np.float32), sig
